# revision 1
# baseline (speedup 1.0000x reference)
"""Trainium2 Bass kernel for nn_MLPMHA (sparse_attention / squared-ReLU MLP-MHA).

Reference computation (B=4, T=2048, C=1024, QH=4, D=256, S=4C=4096):
    x   = layernorm(residual) * g + b
    q_h = x[:, h*D:(h+1)*D]                     per head h
    k   = w_fc.reshape(S, D)                    keys   (shared across heads)
    v   = w_proj.T.reshape(S, D)                values (shared across heads)
    out = residual + concat_h( relu(q_h @ k.T)^2 @ v )

Equivalent blocked form used here (cc = 0..3 indexes 256-wide column chunks
of w_fc / row chunks of w_proj; all matmuls are plain GEMMs):
    A_{h,cc}  = x_h @ w_fc[:, cc*D:(cc+1)*D].T          (T, C)
    out_h     = sum_cc relu(A_{h,cc})^2 @ w_proj[cc*D:(cc+1)*D, :].T   (T, D)

Sharding: pure data parallel over the 8192 = B*T token rows; each of the 8
cores processes 1024 rows with full (transposed) weights resident in SBUF.

On-core dataflow (fp32 everywhere except matmul operands, which are stored as
float32r so the PE runs at 1 cycle/row instead of fp32's 4; measured accuracy
cost ~1.5e-4 relative):
    phase A: DMA residual rows, LayerNorm (bn_stats), PE-transpose x into
             xT[c, t] layout, fusing the ln_g/ln_b affine into the copy-back.
    phase B: per (h, cc, i-chunk): A^T tile = wfcT_chunk.T @ xT  (PSUM),
             relu^2 via one fused DVE op into pT (SBUF),
             out^T PSUM accumulation over all (cc, i): wprojT_chunk.T @ pT.
    phase C: PE-transpose out^T back to natural layout, add into the
             residual-initialised output buffer, DMA out.
"""

import numpy as np

import concourse.bass as bass
import concourse.tile as tile
from concourse import mybir, bacc
from concourse.bass_utils import run_bass_kernel_spmd
from concourse.masks import make_identity

P = 128
C = 1024
D = 256
QH = 4
NCC = 4          # column chunks of w_fc (S = NCC * C kv entries)
N_CORES = 8
ROWS = 1024      # token rows per core (8192 / 8)
NT = ROWS // P   # 8 row tiles per core
EPS = 1e-5

F32 = mybir.dt.float32
F32R = mybir.dt.float32r
BF16 = mybir.dt.bfloat16

_NC_CACHE = {}

# tuning knobs (A/B tested on hardware)
CONFIG = {
    "lookahead": 1,        # software-pipeline depth for mm2 behind mm1
    "relu_alt": True,      # alternate relu/square engines per block
    "pools": (2, 4, 2),    # psA, psO, psT bufs (psT=0 => share psA slots)
    "mm2_bf16": False,     # run relu^2 + second matmul in bf16
    "pt_bufs": 3,          # ptpool bufs
    "work_bufs": 3,        # work pool bufs
    "defer_epi": True,     # emit head epilogue after next head's first mm1s
    "epi_at": 3,           # block index in next head where epilogue lands
}


def _build_body(tc, resid, wfcT, wprojT, ln_g, ln_b, out, reps, variant='full'):  # noqa: C901
    nc = tc.nc
    import contextlib
    ctx = contextlib.ExitStack()
    with ctx:
        singles = ctx.enter_context(tc.tile_pool(name="singles", bufs=1))
        work = ctx.enter_context(tc.tile_pool(name="work", bufs=CONFIG["work_bufs"]))
        ptpool = ctx.enter_context(tc.tile_pool(name="ptpool", bufs=CONFIG["pt_bufs"]))
        psA = ctx.enter_context(tc.tile_pool(name="psA", bufs=CONFIG["pools"][0], space="PSUM"))
        psO = ctx.enter_context(tc.tile_pool(name="psO", bufs=CONFIG["pools"][1], space="PSUM"))
        if CONFIG["pools"][2]:
            psT = ctx.enter_context(tc.tile_pool(name="psT", bufs=CONFIG["pools"][2], space="PSUM"))
        else:
            psT = psA  # transposes share the psA slots (same tag => same banks)

        # ---- resident tensors -------------------------------------------
        wfcT_sb = singles.tile([P, 8, C], F32R)
        nc.sync.dma_start(wfcT_sb[:], wfcT.rearrange("(o p) i -> p o i", p=P))
        wprojT_sb = singles.tile([P, 8, C],
                                 BF16 if CONFIG["mm2_bf16"] else F32R)
        nc.sync.dma_start(wprojT_sb[:], wprojT.rearrange("(o p) i -> p o i", p=P))
        xT_sb = singles.tile([P, 8, ROWS], F32R)
        out_sb = singles.tile([P, NT, C], F32)
        g_sb = singles.tile([P, 8], F32)
        nc.sync.dma_start(g_sb[:], ln_g.rearrange("(o p) -> p o", p=P))
        b_sb = singles.tile([P, 8], F32)
        nc.sync.dma_start(b_sb[:], ln_b.rearrange("(o p) -> p o", p=P))
        ident = singles.tile([P, P], F32)
        make_identity(nc, ident[:])
        eps_t = singles.tile([P, 1], F32)
        nc.vector.memset(eps_t[:], EPS)
        zero_t = singles.tile([P, 1], F32)
        nc.vector.memset(zero_t[:], 0.0)
        one_t = singles.tile([P, 1], F32)
        nc.vector.memset(one_t[:], 1.0)
        pT_dummy = None
        if variant != 'full':
            # diagnostics-only variants may skip the phases that write these
            pT_dummy = singles.tile([P, ROWS], F32R)
            nc.sync.dma_start(pT_dummy[:], wfcT[0:P, :])
            nc.sync.dma_start(xT_sb[:], wfcT.rearrange("(o p) i -> p o i", p=P))
            nc.vector.memset(out_sb[:], 0.0)

        # ---- phases A/B/C, repeated `reps` times for benchmarking -------
        # (each rep recomputes from the DMA'd inputs and rewrites the same
        # output, so the result stays correct for any reps >= 1).  reps > 1
        # uses a hardware loop so the instruction count stays constant.
        if reps == 1:
            _phase_abc(nc, tc, work, ptpool, psA, psO, psT,
                       resid, out, wfcT_sb, wprojT_sb, xT_sb, out_sb,
                       g_sb, b_sb, ident, eps_t, 0, variant, pT_dummy)
        else:
            hint = (mybir.EngineType.PE, mybir.EngineType.Activation,
                    mybir.EngineType.DVE, mybir.EngineType.SP,
                    mybir.EngineType.Pool)
            with tc.For_i(0, reps, 1, hint_engines=hint):
                _phase_abc(nc, tc, work, ptpool, psA, psO, psT,
                           resid, out, wfcT_sb, wprojT_sb, xT_sb, out_sb,
                           g_sb, b_sb, ident, eps_t, 0, variant, pT_dummy)


def _phase_abc(nc, tc, work, ptpool, psA, psO, psT, resid, out,
               wfcT_sb, wprojT_sb, xT_sb, out_sb, g_sb, b_sb, ident,
               eps_t, rep, variant='full', pT_dummy=None):
        # ---- phase A: LayerNorm + transpose into xT ---------------------
        for tt in range([] if variant in ('b_only','mm_only','mm1_only') else range(NT) and range(NT)) if False else (range(0) if variant in ('b_only','mm_only','mm1_only') else range(NT)):
            r_tile = work.tile([P, C], F32, name=f"r_{rep}_{tt}", tag="r_tile")
            nc.sync.dma_start(r_tile[:], resid[tt * P:(tt + 1) * P, :])
            # output starts as the residual; head outputs accumulate into it
            nc.sync.dma_start(out_sb[:, tt, :], resid[tt * P:(tt + 1) * P, :])

            stats = work.tile([P, 2, 6], F32, name=f"st_{rep}_{tt}", tag="stats")
            nc.vector.bn_stats(stats[:, 0, :], r_tile[:, 0:512])
            nc.vector.bn_stats(stats[:, 1, :], r_tile[:, 512:1024])
            mv = work.tile([P, 2], F32, name=f"mv_{rep}_{tt}", tag="mv")
            nc.vector.bn_aggr(mv[:], stats[:])
            # mv[:,1] = 1/sqrt(var + eps)
            nc.scalar.activation(mv[:, 1:2], mv[:, 1:2],
                                 mybir.ActivationFunctionType.Sqrt,
                                 bias=eps_t[:], scale=1.0)
            nc.vector.reciprocal(mv[:, 1:2], mv[:, 1:2])
            # nmr = -mu * rstd, so xn = r*rstd + nmr can run on the (idle)
            # scalar engine, shortening the serial DVE prologue
            nmr = work.tile([P, 1], F32, name=f"nmr_{rep}_{tt}", tag="nmr")
            nc.vector.tensor_scalar(out=nmr[:], in0=mv[:, 0:1],
                                    scalar1=mv[:, 1:2], scalar2=-1.0,
                                    op0=mybir.AluOpType.mult,
                                    op1=mybir.AluOpType.mult)
            xn = work.tile([P, C], F32, name=f"xn_{rep}_{tt}", tag="xn")
            nc.scalar.activation(xn[:], r_tile[:],
                                 mybir.ActivationFunctionType.Identity,
                                 bias=nmr[:], scale=mv[:, 1:2])
            for och in range(8):
                _ptag = "psa" if psT is psA else "pst"
                pst = psT.tile([P, 512], F32, name=f"psx_{rep}_{tt}_{och}", tag=_ptag)[:, :P]
                nc.tensor.transpose(pst[:], xn[:, och * P:(och + 1) * P], ident[:])
                # fused (x_hat * g + b) on the transposed layout (g,b are
                # per-partition scalars there); runs on ACT to keep DVE free
                nc.scalar.activation(xT_sb[:, och, tt * P:(tt + 1) * P], pst[:],
                                     mybir.ActivationFunctionType.Identity,
                                     bias=b_sb[:, och:och + 1],
                                     scale=g_sb[:, och:och + 1])

        # ---- phase B: the two big matmuls per (head, cc, i-chunk) -------
        # Software-pipelined by one block: mm2 for block k is emitted after
        # mm1 for block k+1, so the in-order PE queue never waits on the
        # ACT-relu / DVE-square chain that produces pT.
        if True:
            pending_epi = [None]

            def _flush_epi():
                if pending_epi[0] is not None:
                    pending_epi[0]()
                    pending_epi[0] = None

            for h in range(QH):
                po = [[psO.tile([P, 512], F32, name=f"po_{rep}_{h}_{dd}_{tch}",
                                tag="po")
                       for tch in range(2)] for dd in range(2)]

                def _mm1_into(cc, ich, pTx):
                    isl = slice(ich * P, (ich + 1) * P)
                    for tch in range(2):
                        ps = psA.tile([P, 512], F32,
                                      name=f"psaq_{rep}_{h}_{cc}_{ich}_{tch}",
                                      tag="psa")
                        tsl = slice(tch * 512, (tch + 1) * 512)
                        nc.tensor.matmul(ps[:], wfcT_sb[:, cc * 2 + 0, isl],
                                         xT_sb[:, h * 2 + 0, tsl],
                                         start=True, stop=False)
                        nc.tensor.matmul(ps[:], wfcT_sb[:, cc * 2 + 1, isl],
                                         xT_sb[:, h * 2 + 1, tsl],
                                         start=False, stop=True)

                def _mm1(cc, ich):
                    pT = ptpool.tile([P, ROWS],
                                     BF16 if CONFIG["mm2_bf16"] else F32R,
                                     name=f"pT_{rep}_{h}_{cc}_{ich}", tag="pT")
                    isl = slice(ich * P, (ich + 1) * P)
                    for tch in range(2):
                        ps = psA.tile([P, 512], F32,
                                      name=f"psa_{rep}_{h}_{cc}_{ich}_{tch}",
                                      tag="psa")
                        tsl = slice(tch * 512, (tch + 1) * 512)
                        nc.tensor.matmul(ps[:],
                                         wfcT_sb[:, cc * 2 + 0, isl],
                                         xT_sb[:, h * 2 + 0, tsl],
                                         start=True, stop=False)
                        nc.tensor.matmul(ps[:],
                                         wfcT_sb[:, cc * 2 + 1, isl],
                                         xT_sb[:, h * 2 + 1, tsl],
                                         start=False, stop=True)
                        # pT = relu(ps)^2 in two passes; engines may
                        # alternate per block to balance ACT vs DVE load
                        rl = work.tile([P, 512],
                                       BF16 if CONFIG["mm2_bf16"] else F32,
                                       name=f"rl_{rep}_{h}_{cc}_{ich}_{tch}",
                                       tag="rl")
                        if (not CONFIG["relu_alt"]) or (ich + tch) % 2 == 0:
                            nc.scalar.activation(rl[:], ps[:],
                                                 mybir.ActivationFunctionType.Relu)
                            nc.vector.tensor_mul(out=pT[:, tsl],
                                                 in0=rl[:], in1=rl[:])
                        else:
                            nc.vector.tensor_scalar_max(out=rl[:], in0=ps[:],
                                                        scalar1=0.0)
                            nc.scalar.activation(pT[:, tsl], rl[:],
                                                 mybir.ActivationFunctionType.Square)
                    return pT

                def _mm2(cc, ich, pT):
                    first = (cc == 0 and ich == 0)
                    last = (cc == NCC - 1 and ich == 7)
                    for dd in range(2):
                        wsl = slice(cc * D + dd * P, cc * D + (dd + 1) * P)
                        for tch in range(2):
                            tsl = slice(tch * 512, (tch + 1) * 512)
                            nc.tensor.matmul(po[dd][tch][:],
                                             wprojT_sb[:, ich, wsl],
                                             pT[:, tsl],
                                             start=first, stop=last)

                pending = []
                nblk = 0
                for cc in range(NCC):
                    for ich in range(8):
                        nblk += 1
                        if nblk == CONFIG["epi_at"]:
                            # previous head's epilogue lands here, hidden
                            # behind this head's first two mm1 blocks
                            _flush_epi()
                        if variant == 'mm_only':
                            # detached: mm2 reads a pre-set dummy, so PE runs
                            # the pure matmul stream with no DVE/ACT deps
                            _mm2(cc, ich, pT_dummy)
                            _mm1_into(cc, ich, None)
                            continue
                        pT = _mm1(cc, ich)
                        if variant == 'mm1_only':
                            continue
                        pending.append((cc, ich, pT))
                        # 2-block lookahead: mm2 for block k issues after the
                        # mm1s of blocks k+1 and k+2, hiding the relu/square
                        # latency from the in-order PE queue
                        if len(pending) > CONFIG["lookahead"]:
                            _mm2(*pending.pop(0))
                for args in pending:
                    _mm2(*args)
                # epilogue for head h: transpose out^T back, add into out_sb
                def _epilogue(h=h, po=po):
                  for dd in range(2 if variant in ('full','b_only') else 0):
                      for tch in range(2):
                          oc = work.tile([P, 512], F32, name=f"oc_{rep}_{h}_{dd}_{tch}",
                                         tag="oc")
                          nc.scalar.activation(oc[:], po[dd][tch][:],
                                               mybir.ActivationFunctionType.Identity)
                          for ts4 in range(4):
                              _ptag = "psa" if psT is psA else "pst"
                              pst = psT.tile([P, 512], F32,
                                             name=f"pso_{rep}_{h}_{dd}_{tch}_{ts4}",
                                             tag=_ptag)[:, :P]
                              nc.tensor.transpose(pst[:], oc[:, ts4 * P:(ts4 + 1) * P],
                                                  ident[:])
                              tt = tch * 4 + ts4
                              csl = slice(h * D + dd * P, h * D + (dd + 1) * P)
                              nc.vector.tensor_add(out=out_sb[:, tt, csl],
                                                   in0=out_sb[:, tt, csl],
                                                   in1=pst[:])

                if CONFIG["defer_epi"]:
                    pending_epi[0] = _epilogue
                else:
                    _epilogue()
            _flush_epi()

        # ---- phase C: store --------------------------------------------
        for tt in range(NT if variant in ('full','b_only') else 0):
            nc.sync.dma_start(out[tt * P:(tt + 1) * P, :], out_sb[:, tt, :])
        return


def build_nc(reps=1, variant='full'):
    key = (reps, variant, str(sorted(CONFIG.items())))
    if key in _NC_CACHE:
        return _NC_CACHE[key]
    nc = bacc.Bacc("TRN2", target_bir_lowering=False, debug=False,
                   num_devices=N_CORES)
    resid = nc.dram_tensor("residual", [ROWS, C], F32, kind="ExternalInput").ap()
    wfcT = nc.dram_tensor("w_fcT", [C, C], F32R, kind="ExternalInput").ap()
    wprojT = nc.dram_tensor("w_projT", [C, C],
                            BF16 if CONFIG["mm2_bf16"] else F32R,
                            kind="ExternalInput").ap()
    ln_g = nc.dram_tensor("ln_g", [C], F32, kind="ExternalInput").ap()
    ln_b = nc.dram_tensor("ln_b", [C], F32, kind="ExternalInput").ap()
    out = nc.dram_tensor("out", [ROWS, C], F32, kind="ExternalOutput").ap()
    with tile.TileContext(nc) as tc:
        _build_body(tc, resid, wfcT, wprojT, ln_g, ln_b, out, reps, variant)
    nc.compile()
    _NC_CACHE[key] = nc
    return nc


def _in_maps(residual, w_fc, w_proj, ln_g, ln_b):
    resid2d = np.ascontiguousarray(residual.reshape(-1, C))
    wfcT = np.ascontiguousarray(w_fc.T)
    wprojT = np.ascontiguousarray(w_proj.T)
    if CONFIG["mm2_bf16"]:
        import ml_dtypes
        wprojT = wprojT.astype(ml_dtypes.bfloat16)
    ln_g = np.ascontiguousarray(ln_g)
    ln_b = np.ascontiguousarray(ln_b)
    return [
        {"residual": resid2d[i * ROWS:(i + 1) * ROWS],
         "w_fcT": wfcT, "w_projT": wprojT, "ln_g": ln_g, "ln_b": ln_b}
        for i in range(N_CORES)
    ]


def run_on_cores(inputs, reps=1):
    nc = build_nc(reps)
    in_maps = _in_maps(**inputs)
    return run_bass_kernel_spmd(nc, in_maps, core_ids=list(range(N_CORES)))


def kernel(residual, w_fc, w_proj, ln_g, ln_b):
    B, T, Cx = residual.shape
    res = run_on_cores(dict(residual=residual, w_fc=w_fc, w_proj=w_proj,
                            ln_g=ln_g, ln_b=ln_b))
    out = np.concatenate([r["out"] for r in res.results], axis=0)
    return out.reshape(B, T, Cx).astype(np.float32)



# revision 5
# speedup vs baseline: 1.1710x; 1.1710x over previous
"""Trainium2 Bass kernel for nn_MLPMHA (sparse_attention / squared-ReLU MLP-MHA).

Reference computation (B=4, T=2048, C=1024, QH=4, D=256, S=4C=4096):
    x   = layernorm(residual) * g + b
    q_h = x[:, h*D:(h+1)*D]                     per head h
    k   = w_fc.reshape(S, D)                    keys   (shared across heads)
    v   = w_proj.T.reshape(S, D)                values (shared across heads)
    out = residual + concat_h( relu(q_h @ k.T)^2 @ v )

Equivalent blocked form used here (cc = 0..3 indexes 256-wide column chunks
of w_fc / row chunks of w_proj; all matmuls are plain GEMMs):
    A_{h,cc}  = x_h @ w_fc[:, cc*D:(cc+1)*D].T          (T, C)
    out_h     = sum_cc relu(A_{h,cc})^2 @ w_proj[cc*D:(cc+1)*D, :].T   (T, D)

Sharding: pure data parallel over the 8192 = B*T token rows; each of the 8
cores processes 1024 rows with full (transposed) weights resident in SBUF.

On-core dataflow (PSUM accumulation fp32; matmul operands bf16 by default —
1 cycle/row on the PE like f32r, but half the SBUF/DMA traffic, FWL weight
loads, 2x DVE throughput, and 1-cycle PE transposes; measured accuracy cost
~2e-3 relative vs the 2e-2 gate):
    phase A: DMA residual rows into a persistent buffer, LayerNorm
             (bn_stats), PE-transpose x into xT[c, t] layout (bf16), fusing
             the ln_g/ln_b affine into the copy-back (ACT/DVE alternating).
    phase B: per (h, cc, i-chunk): A^T tile = wfcT_chunk.T @ xT  (PSUM),
             relu^2 via ACT-relu + DVE-square (or one DVE STT op every
             `stt_every`-th tile), out^T PSUM accumulation over all (cc, i).
    phase C: PE-transpose out^T back to natural layout, add into the
             residual buffer in place, DMA out.
"""

import numpy as np

import concourse.bass as bass
import concourse.tile as tile
from concourse import mybir, bacc
from concourse.bass_utils import run_bass_kernel_spmd
from concourse.masks import make_identity

P = 128
C = 1024
D = 256
QH = 4
NCC = 4          # column chunks of w_fc (S = NCC * C kv entries)
N_CORES = 8
ROWS = 1024      # token rows per core (8192 / 8)
NT = ROWS // P   # 8 row tiles per core
EPS = 1e-5

F32 = mybir.dt.float32
F32R = mybir.dt.float32r
BF16 = mybir.dt.bfloat16

_NC_CACHE = {}

# tuning knobs (A/B tested on hardware)
CONFIG = {
    "lookahead": 1,        # software-pipeline depth for mm2 behind mm1
    "pools": (2, 4, 2),    # psA, psO, psT bufs (psT=0 => share psA slots)
    "pt_bufs": 3,          # ptpool bufs
    "work_bufs": 3,        # work pool bufs
    "defer_epi": True,     # emit head epilogue after next head's first mm1s
    "epi_at": 3,           # block index in next head where epilogue lands
    "bf16": True,          # bf16 matmul operands (weights, xT, pT) + transposes
    "stt_every": 0,        # every k-th relu^2 tile as one DVE STT op (0=off;
                           # broken in the current neff lowering)
    "mix": "XXXXZ",        # relu^2 recipe cycle: X = ACT-relu + DVE-square,
                           # Z = DVE-relu + DVE-square, Y = DVE-relu + ACT-sq
    "xn_dve": True,        # layernorm-apply on DVE instead of ACT
    "aff_split": True,     # alternate phase-A copy-backs between ACT and DVE
    "epi_dmat": False,     # epilogue transposes via DMA xbar instead of PE
    "w_reorder": False,    # mm1 issue order sharing stationary weights
}


def _build_body(tc, resid, wfcT, wprojT, ln_g, ln_b, out, reps, variant='full'):  # noqa: C901
    nc = tc.nc
    import contextlib
    cfg = CONFIG
    MDT = BF16 if cfg["bf16"] else F32R    # matmul operand dtype
    ctx = contextlib.ExitStack()
    with ctx:
        singles = ctx.enter_context(tc.tile_pool(name="singles", bufs=1))
        work = ctx.enter_context(tc.tile_pool(name="work", bufs=cfg["work_bufs"]))
        ptpool = ctx.enter_context(tc.tile_pool(name="ptpool", bufs=cfg["pt_bufs"]))
        psA = ctx.enter_context(tc.tile_pool(name="psA", bufs=cfg["pools"][0], space="PSUM"))
        psO = ctx.enter_context(tc.tile_pool(name="psO", bufs=cfg["pools"][1], space="PSUM"))
        if cfg["pools"][2]:
            psT = ctx.enter_context(tc.tile_pool(name="psT", bufs=cfg["pools"][2], space="PSUM"))
        else:
            psT = psA  # transposes share the psA slots (same tag => same banks)

        # ---- resident tensors -------------------------------------------
        wfcT_sb = singles.tile([P, 8, C], MDT)
        nc.sync.dma_start(wfcT_sb[:], wfcT.rearrange("(o p) i -> p o i", p=P))
        wprojT_sb = singles.tile([P, 8, C], MDT)
        nc.sync.dma_start(wprojT_sb[:], wprojT.rearrange("(o p) i -> p o i", p=P))
        xT_sb = singles.tile([P, 8, ROWS], MDT)
        resid_sb = singles.tile([P, NT, C], F32)   # residual in, output out
        g_sb = singles.tile([P, 8], F32)
        nc.sync.dma_start(g_sb[:], ln_g.rearrange("(o p) -> p o", p=P))
        b_sb = singles.tile([P, 8], F32)
        nc.sync.dma_start(b_sb[:], ln_b.rearrange("(o p) -> p o", p=P))
        ident = singles.tile([P, P], BF16 if cfg["bf16"] else F32)
        make_identity(nc, ident[:])
        eps_t = singles.tile([P, 1], F32)
        nc.vector.memset(eps_t[:], EPS)
        pT_dummy = None
        if variant != 'full':
            # diagnostics-only variants may skip the phases that write these
            pT_dummy = singles.tile([P, ROWS], MDT)
            nc.sync.dma_start(pT_dummy[:], wfcT[0:P, :])
            nc.sync.dma_start(xT_sb[:], wfcT.rearrange("(o p) i -> p o i", p=P))
            nc.vector.memset(resid_sb[:], 0.0)

        # ---- phases A/B/C, repeated `reps` times for benchmarking -------
        # (each rep recomputes from the DMA'd inputs and rewrites the same
        # output, so the result stays correct for any reps >= 1).  reps > 1
        # uses a hardware loop so the instruction count stays constant.
        if reps == 1:
            _phase_abc(nc, tc, work, ptpool, psA, psO, psT,
                       resid, out, wfcT_sb, wprojT_sb, xT_sb, resid_sb,
                       g_sb, b_sb, ident, eps_t, 0, variant, pT_dummy)
        else:
            hint = (mybir.EngineType.PE, mybir.EngineType.Activation,
                    mybir.EngineType.DVE, mybir.EngineType.SP,
                    mybir.EngineType.Pool)
            with tc.For_i(0, reps, 1, hint_engines=hint):
                _phase_abc(nc, tc, work, ptpool, psA, psO, psT,
                           resid, out, wfcT_sb, wprojT_sb, xT_sb, resid_sb,
                           g_sb, b_sb, ident, eps_t, 0, variant, pT_dummy)


def _phase_abc(nc, tc, work, ptpool, psA, psO, psT, resid, out,
               wfcT_sb, wprojT_sb, xT_sb, resid_sb, g_sb, b_sb, ident,
               eps_t, rep, variant='full', pT_dummy=None):
    cfg = CONFIG
    MDT = BF16 if cfg["bf16"] else F32R
    TDT = BF16 if cfg["bf16"] else F32     # transpose dtype
    _ptag = "psa" if psT is psA else "pst"
    skip_a = variant in ('b_only', 'mm_only', 'mm1_only')

    # ---- phase A: LayerNorm + transpose into xT -------------------------
    for tt in range(0 if skip_a else NT):
        nc.sync.dma_start(resid_sb[:, tt, :], resid[tt * P:(tt + 1) * P, :])
        r_tile = resid_sb[:, tt, :]

        stats = work.tile([P, 2, 6], F32, name=f"st_{rep}_{tt}", tag="stats")
        nc.vector.bn_stats(stats[:, 0, :], r_tile[:, 0:512])
        nc.vector.bn_stats(stats[:, 1, :], r_tile[:, 512:1024])
        mv = work.tile([P, 2], F32, name=f"mv_{rep}_{tt}", tag="mv")
        nc.vector.bn_aggr(mv[:], stats[:])
        # mv[:,1] = 1/sqrt(var + eps)
        nc.scalar.activation(mv[:, 1:2], mv[:, 1:2],
                             mybir.ActivationFunctionType.Sqrt,
                             bias=eps_t[:], scale=1.0)
        nc.vector.reciprocal(mv[:, 1:2], mv[:, 1:2])
        # nmr = -mu * rstd, so xn = r*rstd + nmr
        nmr = work.tile([P, 1], F32, name=f"nmr_{rep}_{tt}", tag="nmr")
        nc.vector.tensor_scalar(out=nmr[:], in0=mv[:, 0:1],
                                scalar1=mv[:, 1:2], scalar2=-1.0,
                                op0=mybir.AluOpType.mult,
                                op1=mybir.AluOpType.mult)
        xn = work.tile([P, C], TDT, name=f"xn_{rep}_{tt}", tag="xn")
        if cfg["xn_dve"]:
            nc.vector.tensor_scalar(out=xn[:], in0=r_tile,
                                    scalar1=mv[:, 1:2], scalar2=nmr[:],
                                    op0=mybir.AluOpType.mult,
                                    op1=mybir.AluOpType.add)
        else:
            nc.scalar.activation(xn[:], r_tile,
                                 mybir.ActivationFunctionType.Identity,
                                 bias=nmr[:], scale=mv[:, 1:2])
        for och in range(8):
            pst = psT.tile([P, 512], TDT, name=f"psx_{rep}_{tt}_{och}",
                           tag=_ptag)[:, :P]
            nc.tensor.transpose(pst[:], xn[:, och * P:(och + 1) * P], ident[:])
            # fused (x_hat * g + b) on the transposed layout (g,b are
            # per-partition scalars there); ACT/DVE alternate to balance
            dst = xT_sb[:, och, tt * P:(tt + 1) * P]
            if cfg["aff_split"] and och % 2 == 1:
                nc.vector.tensor_scalar(out=dst, in0=pst[:],
                                        scalar1=g_sb[:, och:och + 1],
                                        scalar2=b_sb[:, och:och + 1],
                                        op0=mybir.AluOpType.mult,
                                        op1=mybir.AluOpType.add)
            else:
                nc.scalar.activation(dst, pst[:],
                                     mybir.ActivationFunctionType.Identity,
                                     bias=b_sb[:, och:och + 1],
                                     scale=g_sb[:, och:och + 1])

    # ---- phase B: the two big matmuls per (head, cc, i-chunk) -----------
    # Software-pipelined: mm2 for block k is emitted `lookahead` blocks
    # after mm1 for block k, so the in-order PE queue never waits on the
    # relu^2 chain that produces pT.
    pending_epi = [None]

    def _flush_epi():
        if pending_epi[0] is not None:
            pending_epi[0]()
            pending_epi[0] = None

    tile_ctr = [0]

    for h in range(QH):
        po = [[psO.tile([P, 512], F32, name=f"po_{rep}_{h}_{dd}_{tch}",
                        tag="po")
               for tch in range(2)] for dd in range(2)]

        def _mm1_raw(cc, ich):
            """mm1 without the nonlinearity (diagnostic variants)."""
            isl = slice(ich * P, (ich + 1) * P)
            for tch in range(2):
                ps = psA.tile([P, 512], F32,
                              name=f"psaq_{rep}_{h}_{cc}_{ich}_{tch}",
                              tag="psa")
                tsl = slice(tch * 512, (tch + 1) * 512)
                nc.tensor.matmul(ps[:], wfcT_sb[:, cc * 2 + 0, isl],
                                 xT_sb[:, h * 2 + 0, tsl],
                                 start=True, stop=False)
                nc.tensor.matmul(ps[:], wfcT_sb[:, cc * 2 + 1, isl],
                                 xT_sb[:, h * 2 + 1, tsl],
                                 start=False, stop=True)

        def _mm1(cc, ich):
            pT = ptpool.tile([P, ROWS], MDT,
                             name=f"pT_{rep}_{h}_{cc}_{ich}", tag="pT")
            isl = slice(ich * P, (ich + 1) * P)
            pss = []
            if cfg["w_reorder"]:
                # w0 for both token halves, then w1: consecutive matmuls
                # share the stationary operand
                for tch in range(2):
                    ps = psA.tile([P, 512], F32,
                                  name=f"psa_{rep}_{h}_{cc}_{ich}_{tch}",
                                  tag="psa")
                    pss.append(ps)
                for wi in range(2):
                    for tch in range(2):
                        tsl = slice(tch * 512, (tch + 1) * 512)
                        nc.tensor.matmul(pss[tch][:],
                                         wfcT_sb[:, cc * 2 + wi, isl],
                                         xT_sb[:, h * 2 + wi, tsl],
                                         start=(wi == 0), stop=(wi == 1))
            else:
                for tch in range(2):
                    ps = psA.tile([P, 512], F32,
                                  name=f"psa_{rep}_{h}_{cc}_{ich}_{tch}",
                                  tag="psa")
                    pss.append(ps)
                    tsl = slice(tch * 512, (tch + 1) * 512)
                    nc.tensor.matmul(ps[:],
                                     wfcT_sb[:, cc * 2 + 0, isl],
                                     xT_sb[:, h * 2 + 0, tsl],
                                     start=True, stop=False)
                    nc.tensor.matmul(ps[:],
                                     wfcT_sb[:, cc * 2 + 1, isl],
                                     xT_sb[:, h * 2 + 1, tsl],
                                     start=False, stop=True)
            # pT = relu(ps)^2; recipe per tile from cfg["mix"]
            for tch in range(2):
                ps = pss[tch]
                tsl = slice(tch * 512, (tch + 1) * 512)
                tile_ctr[0] += 1
                if cfg["stt_every"] and tile_ctr[0] % cfg["stt_every"] == 0:
                    nc.vector.scalar_tensor_tensor(
                        out=pT[:, tsl], in0=ps[:], scalar=0.0, in1=ps[:],
                        op0=mybir.AluOpType.max, op1=mybir.AluOpType.mult)
                    continue
                recipe = cfg["mix"][tile_ctr[0] % len(cfg["mix"])]
                rl = work.tile([P, 512], MDT,
                               name=f"rl_{rep}_{h}_{cc}_{ich}_{tch}",
                               tag="rl")
                if recipe == "X":
                    nc.scalar.activation(rl[:], ps[:],
                                         mybir.ActivationFunctionType.Relu)
                    nc.vector.tensor_mul(out=pT[:, tsl], in0=rl[:], in1=rl[:])
                elif recipe == "Z":
                    nc.vector.tensor_scalar_max(out=rl[:], in0=ps[:],
                                                scalar1=0.0)
                    nc.vector.tensor_mul(out=pT[:, tsl], in0=rl[:], in1=rl[:])
                else:  # Y
                    nc.vector.tensor_scalar_max(out=rl[:], in0=ps[:],
                                                scalar1=0.0)
                    nc.scalar.activation(pT[:, tsl], rl[:],
                                         mybir.ActivationFunctionType.Square)
            return pT

        def _mm2(cc, ich, pT):
            first = (cc == 0 and ich == 0)
            last = (cc == NCC - 1 and ich == 7)
            for dd in range(2):
                wsl = slice(cc * D + dd * P, cc * D + (dd + 1) * P)
                for tch in range(2):
                    tsl = slice(tch * 512, (tch + 1) * 512)
                    nc.tensor.matmul(po[dd][tch][:],
                                     wprojT_sb[:, ich, wsl],
                                     pT[:, tsl],
                                     start=first, stop=last)

        pending = []
        nblk = 0
        for cc in range(NCC):
            for ich in range(8):
                nblk += 1
                if nblk == cfg["epi_at"]:
                    # previous head's epilogue lands here, hidden behind
                    # this head's first mm1 blocks
                    _flush_epi()
                if variant == 'mm_only':
                    # detached: mm2 reads a pre-set dummy, so PE runs the
                    # pure matmul stream with no DVE/ACT deps
                    _mm2(cc, ich, pT_dummy)
                    _mm1_raw(cc, ich)
                    continue
                pT = _mm1(cc, ich)
                if variant == 'mm1_only':
                    continue
                pending.append((cc, ich, pT))
                if len(pending) > cfg["lookahead"]:
                    _mm2(*pending.pop(0))
        for args in pending:
            _mm2(*args)

        # epilogue for head h: transpose out^T back, add into resid_sb
        def _epilogue(h=h, po=po):
            if variant not in ('full', 'b_only'):
                return
            for dd in range(2):
                for tch in range(2):
                    oc = work.tile([P, 512], TDT,
                                   name=f"oc_{rep}_{h}_{dd}_{tch}", tag="oc")
                    nc.scalar.activation(oc[:], po[dd][tch][:],
                                         mybir.ActivationFunctionType.Identity)
                    csl0 = h * D + dd * P
                    if cfg["epi_dmat"]:
                        ocT = work.tile([P, 512], TDT,
                                        name=f"ocT_{rep}_{h}_{dd}_{tch}",
                                        tag="ocT")
                        for ts4 in range(4):
                            nc.sync.dma_start_transpose(
                                ocT[:, ts4 * P:(ts4 + 1) * P],
                                oc[:, ts4 * P:(ts4 + 1) * P])
                        for ts4 in range(4):
                            tt = tch * 4 + ts4
                            csl = slice(csl0, csl0 + P)
                            nc.vector.tensor_add(
                                out=resid_sb[:, tt, csl],
                                in0=resid_sb[:, tt, csl],
                                in1=ocT[:, ts4 * P:(ts4 + 1) * P])
                    else:
                        for ts4 in range(4):
                            pst = psT.tile([P, 512], TDT,
                                           name=f"pso_{rep}_{h}_{dd}_{tch}_{ts4}",
                                           tag=_ptag)[:, :P]
                            nc.tensor.transpose(pst[:],
                                                oc[:, ts4 * P:(ts4 + 1) * P],
                                                ident[:])
                            tt = tch * 4 + ts4
                            csl = slice(csl0, csl0 + P)
                            nc.vector.tensor_add(out=resid_sb[:, tt, csl],
                                                 in0=resid_sb[:, tt, csl],
                                                 in1=pst[:])

        if cfg["defer_epi"]:
            pending_epi[0] = _epilogue
        else:
            _epilogue()
    _flush_epi()

    # ---- phase C: store -------------------------------------------------
    for tt in range(NT if variant in ('full', 'b_only') else 0):
        nc.sync.dma_start(out[tt * P:(tt + 1) * P, :], resid_sb[:, tt, :])


def build_nc(reps=1, variant='full'):
    key = (reps, variant, str(sorted(CONFIG.items())))
    if key in _NC_CACHE:
        return _NC_CACHE[key]
    MDT = BF16 if CONFIG["bf16"] else F32R
    nc = bacc.Bacc("TRN2", target_bir_lowering=False, debug=False,
                   num_devices=N_CORES)
    resid = nc.dram_tensor("residual", [ROWS, C], F32, kind="ExternalInput").ap()
    wfcT = nc.dram_tensor("w_fcT", [C, C], MDT, kind="ExternalInput").ap()
    wprojT = nc.dram_tensor("w_projT", [C, C], MDT, kind="ExternalInput").ap()
    ln_g = nc.dram_tensor("ln_g", [C], F32, kind="ExternalInput").ap()
    ln_b = nc.dram_tensor("ln_b", [C], F32, kind="ExternalInput").ap()
    out = nc.dram_tensor("out", [ROWS, C], F32, kind="ExternalOutput").ap()
    with tile.TileContext(nc) as tc:
        _build_body(tc, resid, wfcT, wprojT, ln_g, ln_b, out, reps, variant)
    nc.compile()
    _NC_CACHE[key] = nc
    return nc


def _in_maps(residual, w_fc, w_proj, ln_g, ln_b):
    resid2d = np.ascontiguousarray(residual.reshape(-1, C))
    wfcT = np.ascontiguousarray(w_fc.T)
    wprojT = np.ascontiguousarray(w_proj.T)
    if CONFIG["bf16"]:
        import ml_dtypes
        wfcT = wfcT.astype(ml_dtypes.bfloat16)
        wprojT = wprojT.astype(ml_dtypes.bfloat16)
    ln_g = np.ascontiguousarray(ln_g)
    ln_b = np.ascontiguousarray(ln_b)
    return [
        {"residual": resid2d[i * ROWS:(i + 1) * ROWS],
         "w_fcT": wfcT, "w_projT": wprojT, "ln_g": ln_g, "ln_b": ln_b}
        for i in range(N_CORES)
    ]


def run_on_cores(inputs, reps=1):
    nc = build_nc(reps)
    in_maps = _in_maps(**inputs)
    return run_bass_kernel_spmd(nc, in_maps, core_ids=list(range(N_CORES)))


def kernel(residual, w_fc, w_proj, ln_g, ln_b):
    B, T, Cx = residual.shape
    res = run_on_cores(dict(residual=residual, w_fc=w_fc, w_proj=w_proj,
                            ln_g=ln_g, ln_b=ln_b))
    out = np.concatenate([r["out"] for r in res.results], axis=0)
    return out.reshape(B, T, Cx).astype(np.float32)


# revision 7
# speedup vs baseline: 6.1147x; 5.2218x over previous
"""Trainium2 Bass kernel for nn_MLPMHA (sparse_attention / squared-ReLU MLP-MHA).

Reference computation (B=4, T=2048, C=1024, QH=4, D=256, S=4C=4096):
    x   = layernorm(residual) * g + b
    q_h = x[:, h*D:(h+1)*D]                     per head h
    k   = w_fc.reshape(S, D)                    keys   (shared across heads)
    v   = w_proj.T.reshape(S, D)                values (shared across heads)
    out = residual + concat_h( relu(q_h @ k.T)^2 @ v )

Equivalent blocked form used here (cc = 0..3 indexes 256-wide column chunks
of w_fc / row chunks of w_proj; all matmuls are plain GEMMs):
    A_{h,cc}  = x_h @ w_fc[:, cc*D:(cc+1)*D].T          (T, C)
    out_h     = sum_cc relu(A_{h,cc})^2 @ w_proj[cc*D:(cc+1)*D, :].T   (T, D)

Sharding: pure data parallel over the 8192 = B*T token rows; each of the 8
cores processes 1024 rows with full (transposed) weights resident in SBUF.

On-core dataflow (PSUM accumulation fp32; matmul operands bf16 by default —
1 cycle/row on the PE like f32r, but half the SBUF/DMA traffic, FWL weight
loads, 2x DVE throughput, and 1-cycle PE transposes; measured accuracy cost
~2e-3 relative vs the 2e-2 gate):
    phase A: DMA residual rows into a persistent buffer, LayerNorm
             (bn_stats), PE-transpose x into xT[c, t] layout (bf16), fusing
             the ln_g/ln_b affine into the copy-back (ACT/DVE alternating).
    phase B: per (h, cc, i-chunk): A^T tile = wfcT_chunk.T @ xT  (PSUM),
             relu^2 via ACT-relu + DVE-square (or one DVE STT op every
             `stt_every`-th tile), out^T PSUM accumulation over all (cc, i).
    phase C: PE-transpose out^T back to natural layout, add into the
             residual buffer in place, DMA out.
"""

import numpy as np

import concourse.bass as bass
import concourse.tile as tile
from concourse import mybir, bacc
from concourse.bass_utils import run_bass_kernel_spmd
from concourse.masks import make_identity

P = 128
C = 1024
D = 256
QH = 4
NCC = 4          # column chunks of w_fc (S = NCC * C kv entries)
N_CORES = 8
ROWS = 1024      # token rows per core (8192 / 8)
NT = ROWS // P   # 8 row tiles per core
EPS = 1e-5

F32 = mybir.dt.float32
F32R = mybir.dt.float32r
BF16 = mybir.dt.bfloat16

_NC_CACHE = {}

# tuning knobs (A/B tested on hardware)
CONFIG = {
    "lookahead": 1,        # software-pipeline depth for mm2 behind mm1
    "pools": (2, 4, 2),    # psA, psO, psT bufs (psT=0 => share psA slots)
    "pt_bufs": 3,          # ptpool bufs
    "work_bufs": 3,        # work pool bufs
    "defer_epi": True,     # emit head epilogue after next head's first mm1s
    "epi_at": 3,           # block index in next head where epilogue lands
    "bf16": True,          # bf16 matmul operands (weights, xT, pT) + transposes
    "stt_every": 0,        # every k-th relu^2 tile as one DVE STT op (0=off;
                           # broken in the current neff lowering)
    "mix": "XXXXZ",        # relu^2 recipe cycle: X = ACT-relu + DVE-square,
                           # Z = DVE-relu + DVE-square, Y = DVE-relu + ACT-sq
    "xn_dve": True,        # layernorm-apply on DVE instead of ACT
    "aff_split": True,     # alternate phase-A copy-backs between ACT and DVE
    "epi_dmat": False,     # epilogue transposes via DMA xbar instead of PE
    "w_reorder": False,    # mm1 issue order sharing stationary weights
}


def _build_body(tc, resid, wfcT, wprojT, ln_g, ln_b, out, reps, variant='full'):  # noqa: C901
    nc = tc.nc
    import contextlib
    cfg = CONFIG
    MDT = BF16 if cfg["bf16"] else F32R    # matmul operand dtype
    ctx = contextlib.ExitStack()
    with ctx:
        singles = ctx.enter_context(tc.tile_pool(name="singles", bufs=1))
        work = ctx.enter_context(tc.tile_pool(name="work", bufs=cfg["work_bufs"]))
        ptpool = ctx.enter_context(tc.tile_pool(name="ptpool", bufs=cfg["pt_bufs"]))
        psA = ctx.enter_context(tc.tile_pool(name="psA", bufs=cfg["pools"][0], space="PSUM"))
        psO = ctx.enter_context(tc.tile_pool(name="psO", bufs=cfg["pools"][1], space="PSUM"))
        if cfg["pools"][2]:
            psT = ctx.enter_context(tc.tile_pool(name="psT", bufs=cfg["pools"][2], space="PSUM"))
        else:
            psT = psA  # transposes share the psA slots (same tag => same banks)

        # ---- resident tensors -------------------------------------------
        wfcT_sb = singles.tile([P, 8, C], MDT)
        nc.sync.dma_start(wfcT_sb[:], wfcT.rearrange("(o p) i -> p o i", p=P))
        wprojT_sb = singles.tile([P, 8, C], MDT)
        nc.sync.dma_start(wprojT_sb[:], wprojT.rearrange("(o p) i -> p o i", p=P))
        xT_sb = singles.tile([P, 8, ROWS], MDT)
        resid_sb = singles.tile([P, NT, C], F32)   # residual in, output out
        g_sb = singles.tile([P, 8], F32)
        nc.sync.dma_start(g_sb[:], ln_g.rearrange("(o p) -> p o", p=P))
        b_sb = singles.tile([P, 8], F32)
        nc.sync.dma_start(b_sb[:], ln_b.rearrange("(o p) -> p o", p=P))
        ident = singles.tile([P, P], BF16 if cfg["bf16"] else F32)
        make_identity(nc, ident[:])
        eps_t = singles.tile([P, 1], F32)
        nc.vector.memset(eps_t[:], EPS)
        pT_dummy = None
        if variant != 'full':
            # diagnostics-only variants may skip the phases that write these
            pT_dummy = singles.tile([P, ROWS], MDT)
            nc.sync.dma_start(pT_dummy[:], wfcT[0:P, :])
            nc.sync.dma_start(xT_sb[:], wfcT.rearrange("(o p) i -> p o i", p=P))
            nc.vector.memset(resid_sb[:], 0.0)

        # ---- phases A/B/C, repeated `reps` times for benchmarking -------
        # (each rep recomputes from the DMA'd inputs and rewrites the same
        # output, so the result stays correct for any reps >= 1).  reps > 1
        # uses a hardware loop so the instruction count stays constant.
        if reps == 1:
            _phase_abc(nc, tc, work, ptpool, psA, psO, psT,
                       resid, out, wfcT_sb, wprojT_sb, xT_sb, resid_sb,
                       g_sb, b_sb, ident, eps_t, 0, variant, pT_dummy)
        else:
            hint = (mybir.EngineType.PE, mybir.EngineType.Activation,
                    mybir.EngineType.DVE, mybir.EngineType.SP,
                    mybir.EngineType.Pool)
            with tc.For_i(0, reps, 1, hint_engines=hint):
                _phase_abc(nc, tc, work, ptpool, psA, psO, psT,
                           resid, out, wfcT_sb, wprojT_sb, xT_sb, resid_sb,
                           g_sb, b_sb, ident, eps_t, 0, variant, pT_dummy)


def _phase_abc(nc, tc, work, ptpool, psA, psO, psT, resid, out,
               wfcT_sb, wprojT_sb, xT_sb, resid_sb, g_sb, b_sb, ident,
               eps_t, rep, variant='full', pT_dummy=None):
    cfg = CONFIG
    MDT = BF16 if cfg["bf16"] else F32R
    TDT = BF16 if cfg["bf16"] else F32     # transpose dtype
    _ptag = "psa" if psT is psA else "pst"
    skip_a = variant in ('b_only', 'mm_only', 'mm1_only')

    # ---- phase A: LayerNorm + transpose into xT -------------------------
    # Batched passes instead of a per-tile serial chain, so each engine sees
    # long runs of identical work and tiles pipeline against each other:
    #   pass 1 (per tile): DMA + bn_stats -> stats_all
    #   pass 2 (once):     aggr / sqrt / recip / -mu*rstd over all tiles
    #   pass 3 (per tile): xn = r*rstd + nmr, 8x (PE transpose + affine copy)
    if not skip_a:
        stats_all = work.tile([P, NT, 2, 6], F32, name=f"sta_{rep}",
                              tag="stats", bufs=1)
        mv_all = work.tile([P, NT, 2], F32, name=f"mva_{rep}", tag="mv",
                           bufs=1)
        nmr_all = work.tile([P, NT], F32, name=f"nmra_{rep}", tag="nmr",
                            bufs=1)
        for tt in range(NT):
            nc.sync.dma_start(resid_sb[:, tt, :],
                              resid[tt * P:(tt + 1) * P, :])
            nc.vector.bn_stats(stats_all[:, tt, 0, :],
                               resid_sb[:, tt, 0:512])
            nc.vector.bn_stats(stats_all[:, tt, 1, :],
                               resid_sb[:, tt, 512:1024])
        for tt in range(NT):
            nc.vector.bn_aggr(mv_all[:, tt, :], stats_all[:, tt, :, :])
        # rstd = 1/sqrt(var+eps), nmr = -mu*rstd (xn = r*rstd + nmr)
        nc.scalar.activation(mv_all[:, :, 1], mv_all[:, :, 1],
                             mybir.ActivationFunctionType.Sqrt,
                             bias=eps_t[:], scale=1.0)
        nc.vector.reciprocal(mv_all[:, :, 1], mv_all[:, :, 1])
        nc.vector.tensor_tensor(out=nmr_all[:], in0=mv_all[:, :, 0],
                                in1=mv_all[:, :, 1],
                                op=mybir.AluOpType.mult)
        nc.vector.tensor_scalar_mul(out=nmr_all[:], in0=nmr_all[:],
                                    scalar1=-1.0)
        for tt in range(NT):
            xn = work.tile([P, C], TDT, name=f"xn_{rep}_{tt}", tag="xn")
            if cfg["xn_dve"]:
                nc.vector.tensor_scalar(out=xn[:], in0=resid_sb[:, tt, :],
                                        scalar1=mv_all[:, tt, 1:2],
                                        scalar2=nmr_all[:, tt:tt + 1],
                                        op0=mybir.AluOpType.mult,
                                        op1=mybir.AluOpType.add)
            else:
                nc.scalar.activation(xn[:], resid_sb[:, tt, :],
                                     mybir.ActivationFunctionType.Identity,
                                     bias=nmr_all[:, tt:tt + 1],
                                     scale=mv_all[:, tt, 1:2])
            for och in range(8):
                pst = psT.tile([P, 512], TDT, name=f"psx_{rep}_{tt}_{och}",
                               tag=_ptag)[:, :P]
                nc.tensor.transpose(pst[:], xn[:, och * P:(och + 1) * P],
                                    ident[:])
                # fused (x_hat * g + b) on the transposed layout (g,b are
                # per-partition scalars there); ACT/DVE alternate to balance
                dst = xT_sb[:, och, tt * P:(tt + 1) * P]
                if cfg["aff_split"] and och % 2 == 1:
                    nc.vector.tensor_scalar(out=dst, in0=pst[:],
                                            scalar1=g_sb[:, och:och + 1],
                                            scalar2=b_sb[:, och:och + 1],
                                            op0=mybir.AluOpType.mult,
                                            op1=mybir.AluOpType.add)
                else:
                    nc.scalar.activation(dst, pst[:],
                                         mybir.ActivationFunctionType.Identity,
                                         bias=b_sb[:, och:och + 1],
                                         scale=g_sb[:, och:och + 1])

    # ---- phase B: the two big matmuls per (head, cc, i-chunk) -----------
    # Software-pipelined: mm2 for block k is emitted `lookahead` blocks
    # after mm1 for block k, so the in-order PE queue never waits on the
    # relu^2 chain that produces pT.
    pending_epi = [None]

    def _flush_epi():
        if pending_epi[0] is not None:
            pending_epi[0]()
            pending_epi[0] = None

    tile_ctr = [0]

    for h in range(0 if variant == 'a_only' else QH):
        po = [[psO.tile([P, 512], F32, name=f"po_{rep}_{h}_{dd}_{tch}",
                        tag="po")
               for tch in range(2)] for dd in range(2)]

        def _mm1_raw(cc, ich):
            """mm1 without the nonlinearity (diagnostic variants)."""
            isl = slice(ich * P, (ich + 1) * P)
            for tch in range(2):
                ps = psA.tile([P, 512], F32,
                              name=f"psaq_{rep}_{h}_{cc}_{ich}_{tch}",
                              tag="psa")
                tsl = slice(tch * 512, (tch + 1) * 512)
                nc.tensor.matmul(ps[:], wfcT_sb[:, cc * 2 + 0, isl],
                                 xT_sb[:, h * 2 + 0, tsl],
                                 start=True, stop=False)
                nc.tensor.matmul(ps[:], wfcT_sb[:, cc * 2 + 1, isl],
                                 xT_sb[:, h * 2 + 1, tsl],
                                 start=False, stop=True)

        def _mm1(cc, ich):
            pT = ptpool.tile([P, ROWS], MDT,
                             name=f"pT_{rep}_{h}_{cc}_{ich}", tag="pT")
            isl = slice(ich * P, (ich + 1) * P)
            pss = []
            if cfg["w_reorder"]:
                # w0 for both token halves, then w1: consecutive matmuls
                # share the stationary operand
                for tch in range(2):
                    ps = psA.tile([P, 512], F32,
                                  name=f"psa_{rep}_{h}_{cc}_{ich}_{tch}",
                                  tag="psa")
                    pss.append(ps)
                for wi in range(2):
                    for tch in range(2):
                        tsl = slice(tch * 512, (tch + 1) * 512)
                        nc.tensor.matmul(pss[tch][:],
                                         wfcT_sb[:, cc * 2 + wi, isl],
                                         xT_sb[:, h * 2 + wi, tsl],
                                         start=(wi == 0), stop=(wi == 1))
            else:
                for tch in range(2):
                    ps = psA.tile([P, 512], F32,
                                  name=f"psa_{rep}_{h}_{cc}_{ich}_{tch}",
                                  tag="psa")
                    pss.append(ps)
                    tsl = slice(tch * 512, (tch + 1) * 512)
                    nc.tensor.matmul(ps[:],
                                     wfcT_sb[:, cc * 2 + 0, isl],
                                     xT_sb[:, h * 2 + 0, tsl],
                                     start=True, stop=False)
                    nc.tensor.matmul(ps[:],
                                     wfcT_sb[:, cc * 2 + 1, isl],
                                     xT_sb[:, h * 2 + 1, tsl],
                                     start=False, stop=True)
            # pT = relu(ps)^2; recipe per tile from cfg["mix"]
            for tch in range(2):
                ps = pss[tch]
                tsl = slice(tch * 512, (tch + 1) * 512)
                tile_ctr[0] += 1
                if cfg["stt_every"] and tile_ctr[0] % cfg["stt_every"] == 0:
                    nc.vector.scalar_tensor_tensor(
                        out=pT[:, tsl], in0=ps[:], scalar=0.0, in1=ps[:],
                        op0=mybir.AluOpType.max, op1=mybir.AluOpType.mult)
                    continue
                recipe = cfg["mix"][tile_ctr[0] % len(cfg["mix"])]
                rl = work.tile([P, 512], MDT,
                               name=f"rl_{rep}_{h}_{cc}_{ich}_{tch}",
                               tag="rl")
                if recipe == "X":
                    nc.scalar.activation(rl[:], ps[:],
                                         mybir.ActivationFunctionType.Relu)
                    nc.vector.tensor_mul(out=pT[:, tsl], in0=rl[:], in1=rl[:])
                elif recipe == "Z":
                    nc.vector.tensor_scalar_max(out=rl[:], in0=ps[:],
                                                scalar1=0.0)
                    nc.vector.tensor_mul(out=pT[:, tsl], in0=rl[:], in1=rl[:])
                else:  # Y
                    nc.vector.tensor_scalar_max(out=rl[:], in0=ps[:],
                                                scalar1=0.0)
                    nc.scalar.activation(pT[:, tsl], rl[:],
                                         mybir.ActivationFunctionType.Square)
            return pT

        def _mm2(cc, ich, pT):
            first = (cc == 0 and ich == 0)
            last = (cc == NCC - 1 and ich == 7)
            for dd in range(2):
                wsl = slice(cc * D + dd * P, cc * D + (dd + 1) * P)
                for tch in range(2):
                    tsl = slice(tch * 512, (tch + 1) * 512)
                    nc.tensor.matmul(po[dd][tch][:],
                                     wprojT_sb[:, ich, wsl],
                                     pT[:, tsl],
                                     start=first, stop=last)

        pending = []
        nblk = 0
        for cc in range(NCC):
            for ich in range(8):
                nblk += 1
                if nblk == cfg["epi_at"]:
                    # previous head's epilogue lands here, hidden behind
                    # this head's first mm1 blocks
                    _flush_epi()
                if variant == 'mm_only':
                    # detached: mm2 reads a pre-set dummy, so PE runs the
                    # pure matmul stream with no DVE/ACT deps
                    _mm2(cc, ich, pT_dummy)
                    _mm1_raw(cc, ich)
                    continue
                pT = _mm1(cc, ich)
                if variant == 'mm1_only':
                    continue
                pending.append((cc, ich, pT))
                if len(pending) > cfg["lookahead"]:
                    _mm2(*pending.pop(0))
        for args in pending:
            _mm2(*args)

        # epilogue for head h: transpose out^T back, add into resid_sb
        def _epilogue(h=h, po=po):
            if variant not in ('full', 'b_only'):
                return
            for dd in range(2):
                for tch in range(2):
                    oc = work.tile([P, 512], TDT,
                                   name=f"oc_{rep}_{h}_{dd}_{tch}", tag="oc")
                    nc.scalar.activation(oc[:], po[dd][tch][:],
                                         mybir.ActivationFunctionType.Identity)
                    csl0 = h * D + dd * P
                    if cfg["epi_dmat"]:
                        ocT = work.tile([P, 512], TDT,
                                        name=f"ocT_{rep}_{h}_{dd}_{tch}",
                                        tag="ocT")
                        for ts4 in range(4):
                            nc.sync.dma_start_transpose(
                                ocT[:, ts4 * P:(ts4 + 1) * P],
                                oc[:, ts4 * P:(ts4 + 1) * P])
                        for ts4 in range(4):
                            tt = tch * 4 + ts4
                            csl = slice(csl0, csl0 + P)
                            nc.vector.tensor_add(
                                out=resid_sb[:, tt, csl],
                                in0=resid_sb[:, tt, csl],
                                in1=ocT[:, ts4 * P:(ts4 + 1) * P])
                    else:
                        for ts4 in range(4):
                            pst = psT.tile([P, 512], TDT,
                                           name=f"pso_{rep}_{h}_{dd}_{tch}_{ts4}",
                                           tag=_ptag)[:, :P]
                            nc.tensor.transpose(pst[:],
                                                oc[:, ts4 * P:(ts4 + 1) * P],
                                                ident[:])
                            tt = tch * 4 + ts4
                            csl = slice(csl0, csl0 + P)
                            nc.vector.tensor_add(out=resid_sb[:, tt, csl],
                                                 in0=resid_sb[:, tt, csl],
                                                 in1=pst[:])

        if cfg["defer_epi"]:
            pending_epi[0] = _epilogue
        else:
            _epilogue()
    _flush_epi()

    # ---- phase C: store -------------------------------------------------
    for tt in range(NT if variant in ('full', 'b_only') else 0):
        nc.sync.dma_start(out[tt * P:(tt + 1) * P, :], resid_sb[:, tt, :])


def build_nc(reps=1, variant='full'):
    key = (reps, variant, str(sorted(CONFIG.items())))
    if key in _NC_CACHE:
        return _NC_CACHE[key]
    MDT = BF16 if CONFIG["bf16"] else F32R
    nc = bacc.Bacc("TRN2", target_bir_lowering=False, debug=False,
                   num_devices=N_CORES)
    resid = nc.dram_tensor("residual", [ROWS, C], F32, kind="ExternalInput").ap()
    wfcT = nc.dram_tensor("w_fcT", [C, C], MDT, kind="ExternalInput").ap()
    wprojT = nc.dram_tensor("w_projT", [C, C], MDT, kind="ExternalInput").ap()
    ln_g = nc.dram_tensor("ln_g", [C], F32, kind="ExternalInput").ap()
    ln_b = nc.dram_tensor("ln_b", [C], F32, kind="ExternalInput").ap()
    out = nc.dram_tensor("out", [ROWS, C], F32, kind="ExternalOutput").ap()
    with tile.TileContext(nc) as tc:
        _build_body(tc, resid, wfcT, wprojT, ln_g, ln_b, out, reps, variant)
    nc.compile()
    _NC_CACHE[key] = nc
    return nc


def _in_maps(residual, w_fc, w_proj, ln_g, ln_b):
    resid2d = np.ascontiguousarray(residual.reshape(-1, C))
    wfcT = np.ascontiguousarray(w_fc.T)
    wprojT = np.ascontiguousarray(w_proj.T)
    if CONFIG["bf16"]:
        import ml_dtypes
        wfcT = wfcT.astype(ml_dtypes.bfloat16)
        wprojT = wprojT.astype(ml_dtypes.bfloat16)
    ln_g = np.ascontiguousarray(ln_g)
    ln_b = np.ascontiguousarray(ln_b)
    return [
        {"residual": resid2d[i * ROWS:(i + 1) * ROWS],
         "w_fcT": wfcT, "w_projT": wprojT, "ln_g": ln_g, "ln_b": ln_b}
        for i in range(N_CORES)
    ]


def run_on_cores(inputs, reps=1):
    nc = build_nc(reps)
    in_maps = _in_maps(**inputs)
    return run_bass_kernel_spmd(nc, in_maps, core_ids=list(range(N_CORES)))


def kernel(residual, w_fc, w_proj, ln_g, ln_b):
    B, T, Cx = residual.shape
    res = run_on_cores(dict(residual=residual, w_fc=w_fc, w_proj=w_proj,
                            ln_g=ln_g, ln_b=ln_b))
    out = np.concatenate([r["out"] for r in res.results], axis=0)
    return out.reshape(B, T, Cx).astype(np.float32)


# revision 14
# speedup vs baseline: 8.3863x; 1.3715x over previous
"""Trainium2 Bass kernel for nn_MLPMHA (sparse_attention / squared-ReLU MLP-MHA).

Reference computation (B=4, T=2048, C=1024, QH=4, D=256, S=4C=4096):
    x   = layernorm(residual) * g + b
    q_h = x[:, h*D:(h+1)*D]                     per head h
    k   = w_fc.reshape(S, D)                    keys   (shared across heads)
    v   = w_proj.T.reshape(S, D)                values (shared across heads)
    out = residual + concat_h( relu(q_h @ k.T)^2 @ v )

Equivalent blocked form used here (cc = 0..3 indexes 256-wide column chunks
of w_fc / row chunks of w_proj; all matmuls are plain GEMMs):
    A_{h,cc}  = x_h @ w_fc[:, cc*D:(cc+1)*D].T          (T, C)
    out_h     = sum_cc relu(A_{h,cc})^2 @ w_proj[cc*D:(cc+1)*D, :].T   (T, D)

Sharding: pure data parallel over the 8192 = B*T token rows; each of the 8
cores processes 1024 rows with full (transposed) weights resident in SBUF.

On-core dataflow (PSUM accumulation fp32; matmul operands bf16 by default —
1 cycle/row on the PE like f32r, but half the SBUF/DMA traffic, FWL weight
loads, 2x DVE throughput, and 1-cycle PE transposes; measured accuracy cost
~2e-3 relative vs the 2e-2 gate):
    phase A: DMA residual rows into a persistent buffer, LayerNorm
             (bn_stats), PE-transpose x into xT[c, t] layout (bf16), fusing
             the ln_g/ln_b affine into the copy-back (ACT/DVE alternating).
    phase B: per (h, cc, i-chunk): A^T tile = wfcT_chunk.T @ xT  (PSUM),
             relu^2 via ACT-relu + DVE-square (or one DVE STT op every
             `stt_every`-th tile), out^T PSUM accumulation over all (cc, i).
    phase C: PE-transpose out^T back to natural layout, add into the
             residual buffer in place, DMA out.
"""

import numpy as np

import concourse.bass as bass
import concourse.tile as tile
from concourse import mybir, bacc
from concourse.bass_utils import run_bass_kernel_spmd
from concourse.masks import make_identity

P = 128
C = 1024
D = 256
QH = 4
NCC = 4          # column chunks of w_fc (S = NCC * C kv entries)
N_CORES = 8
ROWS = 1024      # token rows per core (8192 / 8)
NT = ROWS // P   # 8 row tiles per core
EPS = 1e-5

F32 = mybir.dt.float32
F32R = mybir.dt.float32r
BF16 = mybir.dt.bfloat16

_NC_CACHE = {}

# tuning knobs (A/B tested on hardware)
CONFIG = {
    "lookahead": 1,        # software-pipeline depth for mm2 behind mm1
    "pools": (2, 4, 2),    # psA, psO, psT bufs (psT=0 => share psA slots)
    "pt_bufs": 3,          # ptpool bufs
    "work_bufs": 3,        # work pool bufs
    "defer_epi": True,     # emit head epilogue after next head's first mm1s
    "epi_at": 3,           # block index in next head where epilogue lands
    "bf16": True,          # bf16 matmul operands (weights, xT, pT) + transposes
    "stt_every": 0,        # every k-th relu^2 tile as one DVE STT op (0=off;
                           # broken in the current neff lowering)
    "mix": "XXXXZ",        # relu^2 recipe cycle: X = ACT-relu + DVE-square,
                           # Z = DVE-relu + DVE-square, Y = DVE-relu + ACT-sq
    "xn_dve": True,        # layernorm-apply on DVE instead of ACT
    "aff_split": True,     # alternate phase-A copy-backs between ACT and DVE
    "epi_dmat": False,     # epilogue transposes via DMA xbar instead of PE
    "w_reorder": False,    # mm1 issue order sharing stationary weights
}


def _build_body(tc, resid, wfcT, wprojT, g_bc, bias_h, out, reps,
                variant='full'):  # noqa: C901
    nc = tc.nc
    import contextlib
    cfg = CONFIG
    MDT = BF16 if cfg["bf16"] else F32R    # matmul operand dtype
    ctx = contextlib.ExitStack()
    with ctx:
        singles = ctx.enter_context(tc.tile_pool(name="singles", bufs=1))
        work = ctx.enter_context(tc.tile_pool(name="work", bufs=cfg["work_bufs"]))
        ptpool = ctx.enter_context(tc.tile_pool(name="ptpool", bufs=cfg["pt_bufs"]))
        psA = ctx.enter_context(tc.tile_pool(name="psA", bufs=cfg["pools"][0], space="PSUM"))
        psO = ctx.enter_context(tc.tile_pool(name="psO", bufs=cfg["pools"][1], space="PSUM"))
        if cfg["pools"][2]:
            psT = ctx.enter_context(tc.tile_pool(name="psT", bufs=cfg["pools"][2], space="PSUM"))
        else:
            psT = psA  # transposes share the psA slots (same tag => same banks)

        # ---- resident tensors -------------------------------------------
        wfcT_sb = singles.tile([P, 8, C], MDT)
        nc.sync.dma_start(wfcT_sb[:], wfcT.rearrange("(o p) i -> p o i", p=P))
        wprojT_sb = singles.tile([P, 8, C], MDT)
        nc.sync.dma_start(wprojT_sb[:], wprojT.rearrange("(o p) i -> p o i", p=P))
        xT_sb = singles.tile([P, 8, ROWS], MDT)
        resid_sb = singles.tile([P, NT, C], F32)   # residual in, output out
        # ln_g replicated across partitions (host-prepared, matmul dtype)
        gbc_sb = singles.tile([P, C], MDT)
        nc.sync.dma_start(gbc_sb[:], g_bc)
        # per-(head, cc, i-chunk) score bias = ln_b @ w_fc (host-prepared)
        bias_sb = singles.tile([P, QH, NCC, 8], F32)
        nc.sync.dma_start(bias_sb[:],
                          bias_h.rearrange("h n (i p) -> p h n i", p=P))
        ident = singles.tile([P, P], BF16 if cfg["bf16"] else F32)
        make_identity(nc, ident[:])
        eps_t = singles.tile([P, 1], F32)
        nc.vector.memset(eps_t[:], EPS)
        pT_dummy = None
        if variant != 'full':
            # diagnostics-only variants may skip the phases that write these
            pT_dummy = singles.tile([P, ROWS], MDT)
            nc.sync.dma_start(pT_dummy[:], wfcT[0:P, :])
            nc.sync.dma_start(xT_sb[:], wfcT.rearrange("(o p) i -> p o i", p=P))
            nc.vector.memset(resid_sb[:], 0.0)

        # ---- phases A/B/C, repeated `reps` times for benchmarking -------
        # (each rep recomputes from the DMA'd inputs and rewrites the same
        # output, so the result stays correct for any reps >= 1).  reps > 1
        # uses a hardware loop so the instruction count stays constant.
        if reps == 1:
            _phase_abc(nc, tc, work, ptpool, psA, psO, psT,
                       resid, out, wfcT_sb, wprojT_sb, xT_sb, resid_sb,
                       gbc_sb, bias_sb, ident, eps_t, 0, variant, pT_dummy)
        else:
            hint = (mybir.EngineType.PE, mybir.EngineType.Activation,
                    mybir.EngineType.DVE, mybir.EngineType.SP,
                    mybir.EngineType.Pool)
            with tc.For_i(0, reps, 1, hint_engines=hint):
                _phase_abc(nc, tc, work, ptpool, psA, psO, psT,
                           resid, out, wfcT_sb, wprojT_sb, xT_sb, resid_sb,
                           gbc_sb, bias_sb, ident, eps_t, 0, variant, pT_dummy)


def _phase_abc(nc, tc, work, ptpool, psA, psO, psT, resid, out,
               wfcT_sb, wprojT_sb, xT_sb, resid_sb, gbc_sb, bias_sb, ident,
               eps_t, rep, variant='full', pT_dummy=None):
    cfg = CONFIG
    MDT = BF16 if cfg["bf16"] else F32R
    TDT = BF16 if cfg["bf16"] else F32     # transpose dtype
    _ptag = "psa" if psT is psA else "pst"
    skip_a = variant in ('b_only', 'mm_only', 'mm1_only')

    # ---- phase A: LayerNorm + transpose into xT -------------------------
    # Batched passes instead of a per-tile serial chain, so each engine sees
    # long runs of identical work and tiles pipeline against each other:
    #   pass 1 (per tile): DMA + bn_stats -> stats_all
    #   pass 2 (once):     aggr / sqrt / recip / -mu*rstd over all tiles
    #   pass 3 (per tile): xn = r*rstd + nmr, 8x (PE transpose + affine copy)
    if not skip_a:
        stats_all = work.tile([P, NT, 2, 6], F32, name=f"sta_{rep}",
                              tag="stats", bufs=1)
        mv_all = work.tile([P, NT, 2], F32, name=f"mva_{rep}", tag="mv",
                           bufs=1)
        nmr_all = work.tile([P, NT], F32, name=f"nmra_{rep}", tag="nmr",
                            bufs=1)
        for tt in range(NT):
            nc.sync.dma_start(resid_sb[:, tt, :],
                              resid[tt * P:(tt + 1) * P, :])
            nc.vector.bn_stats(stats_all[:, tt, 0, :],
                               resid_sb[:, tt, 0:512])
            nc.vector.bn_stats(stats_all[:, tt, 1, :],
                               resid_sb[:, tt, 512:1024])
        for tt in range(NT):
            nc.vector.bn_aggr(mv_all[:, tt, :], stats_all[:, tt, :, :])
        # rstd = 1/sqrt(var+eps), nmr = -mu*rstd (xn = r*rstd + nmr)
        nc.scalar.activation(mv_all[:, :, 1], mv_all[:, :, 1],
                             mybir.ActivationFunctionType.Sqrt,
                             bias=eps_t[:], scale=1.0)
        nc.vector.reciprocal(mv_all[:, :, 1], mv_all[:, :, 1])
        nc.vector.tensor_tensor(out=nmr_all[:], in0=mv_all[:, :, 0],
                                in1=mv_all[:, :, 1],
                                op=mybir.AluOpType.mult)
        nc.vector.tensor_scalar_mul(out=nmr_all[:], in0=nmr_all[:],
                                    scalar1=-1.0)
        for tt in range(NT):
            xn = work.tile([P, C], TDT, name=f"xn_{rep}_{tt}", tag="xn")
            if cfg["xn_dve"]:
                nc.vector.tensor_scalar(out=xn[:], in0=resid_sb[:, tt, :],
                                        scalar1=mv_all[:, tt, 1:2],
                                        scalar2=nmr_all[:, tt:tt + 1],
                                        op0=mybir.AluOpType.mult,
                                        op1=mybir.AluOpType.add)
            else:
                nc.scalar.activation(xn[:], resid_sb[:, tt, :],
                                     mybir.ActivationFunctionType.Identity,
                                     bias=nmr_all[:, tt:tt + 1],
                                     scale=mv_all[:, tt, 1:2])
            # x_hat * g: ln_g varies along the free (channel) axis here, so
            # one broadcast-multiply handles all 8 chunks (ln_b is folded
            # into the score bias bias_sb = ln_b @ w_fc, applied at the relu)
            nc.vector.tensor_mul(out=xn[:], in0=xn[:], in1=gbc_sb[:])
            # transposes grouped 4-per-PSUM-bank with one grouped copy, to
            # cut the non-PE op count (errata makes small ops expensive)
            for grp in range(2):
                pst = psT.tile([P, 512], TDT, name=f"psx_{rep}_{tt}_{grp}",
                               tag=_ptag)
                for k in range(4):
                    och = grp * 4 + k
                    nc.tensor.transpose(pst[:, k * P:(k + 1) * P],
                                        xn[:, och * P:(och + 1) * P],
                                        ident[:])
                dst = xT_sb[:, grp * 4:(grp + 1) * 4, tt * P:(tt + 1) * P]
                if cfg["aff_split"] and grp % 2 == 1:
                    nc.vector.tensor_copy(dst, pst.rearrange("p (k c) -> p k c", k=4))
                else:
                    nc.scalar.activation(dst,
                                         pst.rearrange("p (k c) -> p k c", k=4),
                                         mybir.ActivationFunctionType.Identity)

    # ---- phase B: the two big matmuls per (head, cc, i-chunk) -----------
    # Software-pipelined: mm2 for block k is emitted `lookahead` blocks
    # after mm1 for block k, so the in-order PE queue never waits on the
    # relu^2 chain that produces pT.
    pending_epi = [None]

    def _flush_epi():
        if pending_epi[0] is not None:
            pending_epi[0]()
            pending_epi[0] = None

    tile_ctr = [0]

    for h in range(0 if variant == 'a_only' else QH):
        po = [[psO.tile([P, 512], F32, name=f"po_{rep}_{h}_{dd}_{tch}",
                        tag="po")
               for tch in range(2)] for dd in range(2)]

        def _mm1_raw(cc, ich):
            """mm1 without the nonlinearity (diagnostic variants)."""
            isl = slice(ich * P, (ich + 1) * P)
            for tch in range(2):
                ps = psA.tile([P, 512], F32,
                              name=f"psaq_{rep}_{h}_{cc}_{ich}_{tch}",
                              tag="psa")
                tsl = slice(tch * 512, (tch + 1) * 512)
                nc.tensor.matmul(ps[:], wfcT_sb[:, cc * 2 + 0, isl],
                                 xT_sb[:, h * 2 + 0, tsl],
                                 start=True, stop=False)
                nc.tensor.matmul(ps[:], wfcT_sb[:, cc * 2 + 1, isl],
                                 xT_sb[:, h * 2 + 1, tsl],
                                 start=False, stop=True)

        def _mm1(cc, ich):
            pT = ptpool.tile([P, ROWS], MDT,
                             name=f"pT_{rep}_{h}_{cc}_{ich}", tag="pT")
            isl = slice(ich * P, (ich + 1) * P)
            pss = []
            if cfg["w_reorder"]:
                # w0 for both token halves, then w1: consecutive matmuls
                # share the stationary operand
                for tch in range(2):
                    ps = psA.tile([P, 512], F32,
                                  name=f"psa_{rep}_{h}_{cc}_{ich}_{tch}",
                                  tag="psa")
                    pss.append(ps)
                for wi in range(2):
                    for tch in range(2):
                        tsl = slice(tch * 512, (tch + 1) * 512)
                        nc.tensor.matmul(pss[tch][:],
                                         wfcT_sb[:, cc * 2 + wi, isl],
                                         xT_sb[:, h * 2 + wi, tsl],
                                         start=(wi == 0), stop=(wi == 1))
            else:
                for tch in range(2):
                    ps = psA.tile([P, 512], F32,
                                  name=f"psa_{rep}_{h}_{cc}_{ich}_{tch}",
                                  tag="psa")
                    pss.append(ps)
                    tsl = slice(tch * 512, (tch + 1) * 512)
                    nc.tensor.matmul(ps[:],
                                     wfcT_sb[:, cc * 2 + 0, isl],
                                     xT_sb[:, h * 2 + 0, tsl],
                                     start=True, stop=False)
                    nc.tensor.matmul(ps[:],
                                     wfcT_sb[:, cc * 2 + 1, isl],
                                     xT_sb[:, h * 2 + 1, tsl],
                                     start=False, stop=True)
            # pT = relu(ps + bias)^2; recipe per tile from cfg["mix"]
            # (bias = ln_b @ w_fc chunk, the folded layernorm bias)
            bias_ap = bias_sb[:, h, cc, ich:ich + 1]
            for tch in range(2):
                ps = pss[tch]
                tsl = slice(tch * 512, (tch + 1) * 512)
                tile_ctr[0] += 1
                recipe = cfg["mix"][tile_ctr[0] % len(cfg["mix"])]
                rl = work.tile([P, 512], MDT,
                               name=f"rl_{rep}_{h}_{cc}_{ich}_{tch}",
                               tag="rl")
                if recipe == "X":
                    nc.scalar.activation(rl[:], ps[:],
                                         mybir.ActivationFunctionType.Relu,
                                         bias=bias_ap, scale=1.0)
                    nc.vector.tensor_mul(out=pT[:, tsl], in0=rl[:], in1=rl[:])
                elif recipe == "Z":
                    nc.vector.tensor_scalar(out=rl[:], in0=ps[:],
                                            scalar1=bias_ap, scalar2=0.0,
                                            op0=mybir.AluOpType.add,
                                            op1=mybir.AluOpType.max)
                    nc.vector.tensor_mul(out=pT[:, tsl], in0=rl[:], in1=rl[:])
                else:  # Y
                    nc.vector.tensor_scalar(out=rl[:], in0=ps[:],
                                            scalar1=bias_ap, scalar2=0.0,
                                            op0=mybir.AluOpType.add,
                                            op1=mybir.AluOpType.max)
                    nc.scalar.activation(pT[:, tsl], rl[:],
                                         mybir.ActivationFunctionType.Square)
            return pT

        def _mm2(cc, ich, pT):
            first = (cc == 0 and ich == 0)
            last = (cc == NCC - 1 and ich == 7)
            for dd in range(2):
                wsl = slice(cc * D + dd * P, cc * D + (dd + 1) * P)
                for tch in range(2):
                    tsl = slice(tch * 512, (tch + 1) * 512)
                    nc.tensor.matmul(po[dd][tch][:],
                                     wprojT_sb[:, ich, wsl],
                                     pT[:, tsl],
                                     start=first, stop=last)

        pending = []
        nblk = 0
        for cc in range(NCC):
            for ich in range(8):
                nblk += 1
                if nblk == cfg["epi_at"]:
                    # previous head's epilogue lands here, hidden behind
                    # this head's first mm1 blocks
                    _flush_epi()
                if variant == 'mm_only':
                    # detached: mm2 reads a pre-set dummy, so PE runs the
                    # pure matmul stream with no DVE/ACT deps
                    _mm2(cc, ich, pT_dummy)
                    _mm1_raw(cc, ich)
                    continue
                pT = _mm1(cc, ich)
                if variant == 'mm1_only':
                    continue
                pending.append((cc, ich, pT))
                if len(pending) > cfg["lookahead"]:
                    _mm2(*pending.pop(0))
        for args in pending:
            _mm2(*args)

        # epilogue for head h: transpose out^T back, add into resid_sb
        def _epilogue(h=h, po=po):
            if variant not in ('full', 'b_only'):
                return
            for dd in range(2):
                for tch in range(2):
                    oc = work.tile([P, 512], TDT,
                                   name=f"oc_{rep}_{h}_{dd}_{tch}", tag="oc")
                    nc.scalar.activation(oc[:], po[dd][tch][:],
                                         mybir.ActivationFunctionType.Identity)
                    csl0 = h * D + dd * P
                    csl = slice(csl0, csl0 + P)
                    tts = slice(tch * 4, (tch + 1) * 4)
                    if cfg["epi_dmat"]:
                        ocT = work.tile([P, 512], TDT,
                                        name=f"ocT_{rep}_{h}_{dd}_{tch}",
                                        tag="ocT")
                        for ts4 in range(4):
                            nc.sync.dma_start_transpose(
                                ocT[:, ts4 * P:(ts4 + 1) * P],
                                oc[:, ts4 * P:(ts4 + 1) * P])
                        nc.vector.tensor_add(
                            out=resid_sb[:, tts, csl],
                            in0=resid_sb[:, tts, csl],
                            in1=ocT.rearrange("p (k c) -> p k c", k=4))
                    else:
                        pst = psT.tile([P, 512], TDT,
                                       name=f"pso_{rep}_{h}_{dd}_{tch}",
                                       tag=_ptag)
                        for ts4 in range(4):
                            nc.tensor.transpose(pst[:, ts4 * P:(ts4 + 1) * P],
                                                oc[:, ts4 * P:(ts4 + 1) * P],
                                                ident[:])
                        # one grouped add over the 4 token tiles
                        nc.vector.tensor_add(
                            out=resid_sb[:, tts, csl],
                            in0=resid_sb[:, tts, csl],
                            in1=pst.rearrange("p (k c) -> p k c", k=4))

        if cfg["defer_epi"]:
            pending_epi[0] = _epilogue
        else:
            _epilogue()
    _flush_epi()

    # ---- phase C: store -------------------------------------------------
    for tt in range(NT if variant in ('full', 'b_only') else 0):
        nc.sync.dma_start(out[tt * P:(tt + 1) * P, :], resid_sb[:, tt, :])


def build_nc(reps=1, variant='full'):
    key = (reps, variant, str(sorted(CONFIG.items())))
    if key in _NC_CACHE:
        return _NC_CACHE[key]
    MDT = BF16 if CONFIG["bf16"] else F32R
    nc = bacc.Bacc("TRN2", target_bir_lowering=False, debug=False,
                   num_devices=N_CORES)
    resid = nc.dram_tensor("residual", [ROWS, C], F32, kind="ExternalInput").ap()
    wfcT = nc.dram_tensor("w_fcT", [C, C], MDT, kind="ExternalInput").ap()
    wprojT = nc.dram_tensor("w_projT", [C, C], MDT, kind="ExternalInput").ap()
    g_bc = nc.dram_tensor("g_bc", [P, C], MDT, kind="ExternalInput").ap()
    bias_h = nc.dram_tensor("bias_h", [QH, NCC, C], F32,
                            kind="ExternalInput").ap()
    out = nc.dram_tensor("out", [ROWS, C], F32, kind="ExternalOutput").ap()
    with tile.TileContext(nc) as tc:
        _build_body(tc, resid, wfcT, wprojT, g_bc, bias_h, out, reps, variant)
    nc.compile()
    _NC_CACHE[key] = nc
    return nc


def _in_maps(residual, w_fc, w_proj, ln_g, ln_b):
    resid2d = np.ascontiguousarray(residual.reshape(-1, C))
    wfcT = np.ascontiguousarray(w_fc.T)
    wprojT = np.ascontiguousarray(w_proj.T)
    # ln_g replicated across partitions; ln_b folded into a per-score bias:
    # score[s=(cc,r)] += sum_d ln_b[h*D+d] * w_fc[r, cc*D+d]
    g_bc = np.broadcast_to(np.asarray(ln_g)[None, :], (P, C))
    b4 = np.asarray(ln_b, np.float32).reshape(QH, D)
    wf4 = np.asarray(w_fc, np.float32).reshape(C, NCC, D)
    bias_h = np.einsum("hd,rcd->hcr", b4, wf4).astype(np.float32)
    bias_h = np.ascontiguousarray(bias_h)
    if CONFIG["bf16"]:
        import ml_dtypes
        wfcT = wfcT.astype(ml_dtypes.bfloat16)
        wprojT = wprojT.astype(ml_dtypes.bfloat16)
        g_bc = g_bc.astype(ml_dtypes.bfloat16)
    else:
        g_bc = g_bc.astype(np.float32)
    g_bc = np.ascontiguousarray(g_bc)
    return [
        {"residual": resid2d[i * ROWS:(i + 1) * ROWS],
         "w_fcT": wfcT, "w_projT": wprojT, "g_bc": g_bc, "bias_h": bias_h}
        for i in range(N_CORES)
    ]


def run_on_cores(inputs, reps=1):
    nc = build_nc(reps)
    in_maps = _in_maps(**inputs)
    return run_bass_kernel_spmd(nc, in_maps, core_ids=list(range(N_CORES)))


def kernel(residual, w_fc, w_proj, ln_g, ln_b):
    B, T, Cx = residual.shape
    res = run_on_cores(dict(residual=residual, w_fc=w_fc, w_proj=w_proj,
                            ln_g=ln_g, ln_b=ln_b))
    out = np.concatenate([r["out"] for r in res.results], axis=0)
    return out.reshape(B, T, Cx).astype(np.float32)
